# revision 34
# baseline (speedup 1.0000x reference)
"""Trainium2 Bass kernel for LoRA multi-head attention (B=2, S=2048, D=768, H=12, R=8).

Sharding over 8 cores: (batch, query-half, head-half) -> each core computes
6 heads x 1024 query rows x full 2048 keys, producing a partial (over the
head dimension) of the final merge projection. Host sums the two head-half
partials per (batch, query-half) slice and adds the merge bias.

Host-side folding (all exact):
  - LoRA deltas are rank-8: folded on the host; the device sees adjusted
    streams.
  - bv is constant per head-dim: folded into the output bias as
    bm_eff = bm + bv @ Wm.T.

Engine budget per core (the design):
  - ACT (scalar) engine does ONLY the 96 exp instructions (the hard floor,
    ~100us busy). No copies, no DMA issue on its queue.
  - fp16 everywhere instead of bf16: PE speed is identical and the error
    drops ~10x (8.5e-4 vs 8.2e-3), far under the 2e-2 gate. fp8/DoubleRow
    was measured numerically infeasible for this gate (3-6e-2).
  - scores pack the two heads of a pair into the 64-row PE halves
    (row-tiled matmuls overlap on the array).
  - att@v keeps v stationary (N=512 streams; the flipped att-stationary
    layout loses to per-matmul LDWEIGHTS cost). Epilogue: den copy +
    fast reciprocal, gpsimd partition-broadcast, one fused
    normalize-multiply into outT (the separate ob-copy is folded in).
  - weight DMAs issue from the gpsimd queue, streams/masks from sync, so
    the ACT queue stays pure.
  - A qh-pair AllGather dedup of the k/v projections was tried and
    reverted: the DRAM-bounce collective path costs far more DMA/latency
    here than the ~30us of PE it saves.
"""

import sys

if "/opt/trn_rl_repo" not in sys.path:
    sys.path.insert(0, "/opt/trn_rl_repo")

import numpy as np

import concourse.tile as tile
from concourse import bacc, mybir
from concourse.bass_utils import run_bass_kernel_spmd

F32 = mybir.dt.float32
F16 = mybir.dt.float16
EXP = mybir.ActivationFunctionType.Exp
MULT = mybir.AluOpType.mult

B, S, D, H, R = 2, 2048, 768, 12, 8
DK = D // H  # 64
NCORES = 8
HPC = 6            # heads per core
HDIM = HPC * DK    # 384: head-slice width per core
QR = S // 2        # 1024 query rows per core
SC = 512           # streaming chunk (s dimension)
NSC = S // SC      # 4
NQC = QR // SC     # 2 query chunks per core
NKT = S // 128     # 16 key tiles
DO = D // 128      # 6 d-chunks

# of every 16 (kt-pair, head) mask multiplies, this many route to gpsimd
GPS_MASK = 2

_CACHE = {}


def _build_kernel():
    nc = bacc.Bacc("TRN2", target_bir_lowering=False, debug=False,
                   enable_asserts=True, num_devices=NCORES)

    def din(name, shape, dt=F16):
        return nc.dram_tensor(name, shape, dt, kind="ExternalInput").ap()

    qT_d = din("qT", [D, S])
    kT_d = din("kT", [D, S])
    vTh_d = din("vTh", [D, QR])
    maskT_d = din("maskT", [S, QR])
    wqt_d = din("wqt", [D, HDIM])
    wkt_d = din("wkt", [D, HDIM])
    wvt_d = din("wvt", [D, HDIM])
    wmt_d = din("wmt", [HDIM, D])
    bqc_d = din("bqc", [128, 3], F32)
    bkc_d = din("bkc", [128, 3], F32)
    out_d = nc.dram_tensor("out", [QR, D], F16, kind="ExternalOutput").ap()

    with tile.TileContext(nc) as tc:
        with (
            tc.tile_pool(name="keep", bufs=1) as keep,
            tc.tile_pool(name="maskp", bufs=2) as maskp,
            tc.tile_pool(name="wa", bufs=1) as wa,
            tc.tile_pool(name="acts", bufs=2) as acts,
            tc.tile_pool(name="bpool", bufs=2) as bpool,
            tc.tile_pool(name="epool", bufs=3) as epool,
            tc.tile_pool(name="npool", bufs=2) as npool,
            tc.tile_pool(name="fpool", bufs=2) as fpool,
            tc.tile_pool(name="psproj", bufs=2, space="PSUM") as psproj,
            tc.tile_pool(name="pss", bufs=2, space="PSUM") as pss,
            tc.tile_pool(name="pso", bufs=2, space="PSUM") as pso,
        ):
            # ---- persistent tiles ----
            wm_sb = keep.tile([128, 3, D], F16)
            kT_sb = keep.tile([128, 3, S], F16)
            qT_sb = keep.tile([128, 3, QR], F16)
            v_aug = keep.tile([128, NKT, HPC, DK + 1], F16)
            nc.vector.memset(v_aug[:, :, :, DK:DK + 1], 1.0)

            # streamed k chunk 0 first on the sync DGE so the PE can start
            kTc0 = acts.tile([128, DO, SC], F16, tag="act", name="kTc0")
            for dp in range(3):
                nc.sync.dma_start(
                    kTc0[:, 2 * dp:2 * dp + 2, :],
                    kT_d.rearrange("(o p) s -> p o s",
                                   p=128)[:, 2 * dp:2 * dp + 2, 0:SC])

            # weights ride the gpsimd DGE (Pool queue is idle; ACT stays pure)
            def wload(name, dram, shape, pat):
                t = wa.tile(shape, F16, name=name)
                nc.gpsimd.dma_start(t[:], dram.rearrange(pat, p=128))
                return t

            wk_sb = wa.tile([128, DO, HDIM], F16, name="wk_sb")
            for _et in range(3):
                _esl = slice(_et * 128, (_et + 1) * 128)
                nc.gpsimd.dma_start(
                    wk_sb[:, :, _esl],
                    wkt_d.rearrange("(o p) e -> p o e", p=128)[:, :, _esl])
            wq_sb = wload("wq_sb", wqt_d, [128, DO, HDIM], "(o p) e -> p o e")
            wv_sb = wload("wv_sb", wvt_d, [128, DO, HDIM], "(o p) e -> p o e")
            bq_sb = wa.tile([128, 3], F32)
            nc.gpsimd.dma_start(bq_sb[:], bqc_d[:])
            bk_sb = wa.tile([128, 3], F32)
            nc.gpsimd.dma_start(bk_sb[:], bkc_d[:])

            # ---- pass 1: k-projection of the OWN key-half (T-form) ----
            kTcs = {0: kTc0}

            def emit_kload(sc):
                ssl = slice(sc * SC, (sc + 1) * SC)
                kTc = acts.tile([128, DO, SC], F16, tag="act", name="kTc")
                for dp in range(3):
                    nc.sync.dma_start(
                        kTc[:, 2 * dp:2 * dp + 2, :],
                        kT_d.rearrange("(o p) s -> p o s",
                                       p=128)[:, 2 * dp:2 * dp + 2, ssl])
                kTcs[sc] = kTc

            def emit_kproj(sc, ets=(0, 1, 2)):
                ssl = slice(sc * SC, (sc + 1) * SC)
                if sc not in kTcs:
                    emit_kload(sc)
                kTc = kTcs[sc]
                for et in ets:
                    esl = slice(et * 128, (et + 1) * 128)
                    ps = psproj.tile([128, 512], F32, tag="pp", name="ps_k")
                    for do in range(DO):
                        nc.tensor.matmul(ps[:], wk_sb[:, do, esl], kTc[:, do, :],
                                         start=(do == 0), stop=(do == DO - 1))
                    nc.vector.tensor_scalar_add(
                        kT_sb[:, et, ssl], ps[:], bk_sb[:, et:et + 1])

            vThcs = {}

            def emit_qproj(qc, ets=(0, 1, 2)):
                qsl = slice(qc * SC, (qc + 1) * SC)
                if qc not in vThcs:
                    vThc = acts.tile([128, DO, SC], F16, tag="act2", name="vThc")
                    for dp in range(3):
                        nc.sync.dma_start(
                            vThc[:, 2 * dp:2 * dp + 2, :],
                            vTh_d.rearrange("(o p) s -> p o s",
                                            p=128)[:, 2 * dp:2 * dp + 2, qsl])
                    vThcs[qc] = vThc
                vThc = vThcs[qc]
                for et in ets:
                    esl = slice(et * 128, (et + 1) * 128)
                    ps = psproj.tile([128, 512], F32, tag="pp", name="ps_q")
                    for do in range(DO):
                        nc.tensor.matmul(ps[:], wq_sb[:, do, esl], vThc[:, do, :],
                                         start=(do == 0), stop=(do == DO - 1))
                    nc.vector.tensor_scalar_add(
                        qT_sb[:, et, qsl], ps[:], bq_sb[:, et:et + 1])

            # ---- phase B helpers ----
            m01s = {}
            atts = {}

            def emit_mask_load(qc):
                qsl = slice(qc * SC, (qc + 1) * SC)
                m01 = maskp.tile([128, NKT, SC], F16, tag="mb", name="m01")
                for kp in range(4):
                    nc.sync.dma_start(
                        m01[:, 4 * kp:4 * kp + 4, :],
                        maskT_d.rearrange(
                            "(o p) q -> p o q", p=128)[:, 4 * kp:4 * kp + 4, qsl])
                m01s[qc] = m01

            def emit_scores_range(qc, p, ktp_lo, ktp_hi):
                # kt-pairs: one exp tile covers 2 key tiles x 2 heads; the
                # mask multiply then runs once per pair (DVE or GPSIMD).
                qsl = slice(qc * SC, (qc + 1) * SC)
                if qc not in m01s:
                    emit_mask_load(qc)
                m01 = m01s[qc]
                if (qc, p) in atts:
                    att = atts[(qc, p)]
                else:
                    att = bpool.tile([128, NKT, 2, SC], F16, tag="att",
                                     name="att")
                    atts[(qc, p)] = att
                for ktp in range(ktp_lo, ktp_hi):
                    exp_t = epool.tile([128, 2, 2, SC], F16, tag="ex",
                                       name="exp_t")
                    for ki in range(2):
                        kt = 2 * ktp + ki
                        ps_s = pss.tile([128, 2, SC], F32, tag="ss", name="ps_s")
                        for j in range(2):
                            po = j * 64
                            nc.tensor.matmul(
                                ps_s[:, j, :],
                                kT_sb[po:po + 64, p, kt * 128:(kt + 1) * 128],
                                qT_sb[po:po + 64, p, qsl],
                                start=True, stop=True)
                        nc.scalar.activation(exp_t[:, ki, :, :], ps_s[:], EXP,
                                             scale=0.125)
                    mk = m01[:, 2 * ktp:2 * ktp + 2, :]
                    for j in range(2):
                        idx = 2 * ktp + j
                        eng = nc.gpsimd if (idx % 16) >= (16 - GPS_MASK) \
                            else nc.vector
                        eng.tensor_mul(
                            att[:, 2 * ktp:2 * ktp + 2, j, :],
                            exp_t[:, :, j, :], mk)

            def emit_scores_pair(qc, p):
                emit_scores_range(qc, p, 0, NKT // 2)

            attv_ps = {}

            def emit_attv_mms(qc, p, kt_lo, kt_hi):
                att = atts[(qc, p)]
                for j in range(2):
                    h = 2 * p + j
                    if (qc, p, j) not in attv_ps:
                        attv_ps[(qc, p, j)] = pso.tile([DK + 1, SC], F32,
                                                       tag="oo", name="ps_o")
                    ps_o = attv_ps[(qc, p, j)]
                    for kt in range(kt_lo, kt_hi):
                        nc.tensor.matmul(ps_o[:], v_aug[:, kt, h, :],
                                         att[:, kt, j, :],
                                         start=(kt == 0), stop=(kt == NKT - 1))

            def emit_attv_epilogue(qc, p):
                atts.pop((qc, p))
                for j in range(2):
                    po = j * 64
                    ps_o = attv_ps.pop((qc, p, j))
                    den_sb = npool.tile([1, SC], F32, tag="den", name="den_sb")
                    nc.vector.tensor_copy(den_sb[:], ps_o[DK:DK + 1, :])
                    r_sb = npool.tile([1, SC], F32, tag="r", name="r_sb")
                    nc.vector.reciprocal_approx_fast(r_sb[:], den_sb[:])
                    bb = npool.tile([64, SC], F32, tag="bb", name="bb")
                    nc.gpsimd.partition_broadcast(bb[:], r_sb[:])
                    if 2 * p + j == 0:
                        _OUTT[qc] = bpool.tile([128, 3, SC], F16, tag="outT",
                                               name="outT")
                    outT_sb = _OUTT[qc]
                    nc.vector.tensor_mul(outT_sb[po:po + 64, p, :],
                                         ps_o[0:DK, :], bb[:])

            def emit_attv_pair(qc, p):
                emit_attv_mms(qc, p, 0, NKT)
                emit_attv_epilogue(qc, p)

            def emit_merge(qc, qts=(0, 1, 2, 3)):
                outT_sb = _OUTT[qc]
                for qt in qts:
                    qtsl = slice(qt * 128, (qt + 1) * 128)
                    for ec in range(2):
                        esl = slice(ec * 384, (ec + 1) * 384)
                        ps_m = psproj.tile([128, 512], F32, tag="pp", name="ps_m")
                        for hp in range(3):
                            nc.tensor.matmul(ps_m[:, :384], outT_sb[:, hp, qtsl],
                                             wm_sb[:, hp, esl],
                                             start=(hp == 0), stop=(hp == 2))
                        fin = fpool.tile([128, 384], F16, tag="fin", name="fin")
                        nc.vector.tensor_copy(fin[:], ps_m[:, :384])
                        nc.sync.dma_start(
                            out_d[qc * SC + qt * 128:qc * SC + (qt + 1) * 128, esl],
                            fin[:])

            # ---- pass 2: v-projection of the OWN key-half ----
            def emit_pass2(sc):
                ssl = slice(sc * SC, (sc + 1) * SC)
                qTc = acts.tile([128, DO, SC], F16, tag="act2", name="qTc")
                for dp in range(3):
                    nc.sync.dma_start(
                        qTc[:, 2 * dp:2 * dp + 2, :],
                        qT_d.rearrange("(o p) s -> p o s",
                                       p=128)[:, 2 * dp:2 * dp + 2, ssl])
                for st in range(4):
                    gst = sc * 4 + st
                    stsl = slice(st * 128, (st + 1) * 128)
                    ps = psproj.tile([128, 512], F32, tag="pp", name="ps_v")
                    for do in range(DO):
                        nc.tensor.matmul(ps[:, :HDIM], qTc[:, do, stsl],
                                         wv_sb[:, do, :],
                                         start=(do == 0), stop=(do == DO - 1))
                    nc.vector.tensor_copy(
                        v_aug[:, gst, :, 0:DK],
                        ps[:, :HDIM].rearrange("p (h d) -> p h d", h=HPC))

            # pass 1 with scores(0,0) interleaved: the exp chain starts
            # during the DMA-paced k-projection.
            emit_kproj(0)
            emit_qproj(0)
            emit_scores_range(0, 0, 0, 2)
            emit_kproj(1)
            emit_scores_range(0, 0, 2, 4)
            emit_kproj(2)
            emit_scores_range(0, 0, 4, 6)
            emit_qproj(1)
            emit_kproj(3)
            emit_scores_range(0, 0, 6, 8)
            emit_pass2(0)
            emit_scores_pair(1, 0)
            emit_pass2(1)
            emit_pass2(2)
            emit_pass2(3)

            # merge weights needed only at the end; gpsimd DGE
            nc.gpsimd.dma_start(wm_sb[:],
                                wmt_d.rearrange("(o p) e -> p o e", p=128))

            # ---- pipelined attention tail ----
            emit_attv_pair(0, 0)
            emit_scores_pair(0, 1)
            emit_attv_pair(1, 0)
            emit_scores_pair(1, 1)
            emit_attv_pair(0, 1)
            emit_scores_pair(0, 2)
            emit_attv_pair(1, 1)
            emit_scores_pair(1, 2)
            emit_attv_pair(0, 2)
            emit_merge(0)
            emit_attv_pair(1, 2)
            emit_merge(1)

    nc.compile()
    return nc


_OUTT = {}


def _shard_inputs(inputs):
    q = np.asarray(inputs["query"], np.float32)
    k = np.asarray(inputs["key"], np.float32)
    v = np.asarray(inputs["value"], np.float32)
    mask = np.asarray(inputs["mask"], np.int32)
    Wq = np.asarray(inputs["Wq"], np.float32)
    Wk = np.asarray(inputs["Wk"], np.float32)
    Wv = np.asarray(inputs["Wv"], np.float32)
    Wm = np.asarray(inputs["Wm"], np.float32)
    bq = np.asarray(inputs["bq"], np.float32)
    bk = np.asarray(inputs["bk"], np.float32)
    Aq = np.asarray(inputs["lora_A_q"], np.float32)
    Bq = np.asarray(inputs["lora_B_q"], np.float32)
    Av = np.asarray(inputs["lora_A_v"], np.float32)
    Bv = np.asarray(inputs["lora_B_v"], np.float32)

    def c(x):
        return np.ascontiguousarray(x)

    def ch(x):
        return np.ascontiguousarray(x.astype(np.float16))

    # fold the rank-8 LoRA deltas on the host (cheap, exact)
    Qadj = q + (q @ Aq) @ Bq      # feeds the v-projection stream
    Vadj = v + (v @ Av) @ Bv      # feeds the q-projection stream

    qT = [ch(Qadj[b].T) for b in range(B)]
    kT = [ch(k[b].T) for b in range(B)]
    vT = [ch(Vadj[b].T) for b in range(B)]
    mT = [ch(mask[b].T) for b in range(B)]
    WqT, WkT, WvT, WmT = ch(Wq.T), ch(Wk.T), ch(Wv.T), ch(Wm.T)

    in_maps = []
    for core in range(NCORES):
        b, qh, hh = core // 4, (core // 2) % 2, core % 2
        hsl = slice(hh * HDIM, (hh + 1) * HDIM)
        qrows = slice(qh * QR, (qh + 1) * QR)
        in_maps.append({
            "qT": qT[b],
            "kT": kT[b],
            "vTh": c(vT[b][:, qrows]),
            "maskT": c(mT[b][:, qrows]),
            "wqt": c(WqT[:, hsl]),
            "wkt": c(WkT[:, hsl]),
            "wvt": c(WvT[:, hsl]),
            "wmt": c(WmT[hsl, :]),
            "bqc": c(bq[hsl].reshape(3, 128).T),
            "bkc": c(bk[hsl].reshape(3, 128).T),
        })
    return in_maps


def _get_nc():
    if "nc" not in _CACHE:
        _CACHE["nc"] = _build_kernel()
    return _CACHE["nc"]


def kernel(**inputs) -> np.ndarray:
    nc = _get_nc()
    in_maps = _shard_inputs(inputs)
    res = run_bass_kernel_spmd(nc, in_maps, core_ids=list(range(NCORES)))
    # bv is constant along keys, so it passes through the softmax average
    # exactly; fold it (and bm) into a host-side output bias.
    bm = np.asarray(inputs["bm"], np.float32)
    bv = np.asarray(inputs["bv"], np.float32)
    Wm = np.asarray(inputs["Wm"], np.float32)
    bm_eff = bm + bv @ Wm.T
    out = np.zeros((B, S, D), np.float32)
    for b in range(B):
        for qh in range(2):
            part = (np.asarray(res.results[b * 4 + qh * 2 + 0]["out"],
                               np.float32)
                    + np.asarray(res.results[b * 4 + qh * 2 + 1]["out"],
                                 np.float32))
            out[b, qh * QR:(qh + 1) * QR, :] = part + bm_eff[None, :]
    return out


# revision 35
# speedup vs baseline: 1.1304x; 1.1304x over previous
"""Trainium2 Bass kernel for LoRA multi-head attention (B=2, S=2048, D=768, H=12, R=8).

Sharding over 8 cores: (batch, query-half, head-half) -> each core computes
6 heads x 1024 query rows x full 2048 keys, producing a partial (over the
head dimension) of the final merge projection. Host sums the two head-half
partials per (batch, query-half) slice and adds the merge bias.

Host-side folding (all exact):
  - LoRA deltas are rank-8: folded on the host; the device sees adjusted
    streams.
  - bv is constant per head-dim: folded into the output bias as
    bm_eff = bm + bv @ Wm.T.

Engine budget per core (the design):
  - ACT (scalar) engine does ONLY the 96 exp instructions (the hard floor,
    ~100us busy). No copies, no DMA issue on its queue.
  - fp16 everywhere instead of bf16: PE speed is identical and the error
    drops ~10x (8.5e-4 vs 8.2e-3), far under the 2e-2 gate. fp8/DoubleRow
    was measured numerically infeasible for this gate (3-6e-2).
  - scores pack the two heads of a pair into the 64-row PE halves
    (row-tiled matmuls overlap on the array).
  - att@v keeps v stationary (N=512 streams; the flipped att-stationary
    layout loses to per-matmul LDWEIGHTS cost). Epilogue: den copy +
    fast reciprocal, gpsimd partition-broadcast, one fused
    normalize-multiply into outT (the separate ob-copy is folded in).
  - weight DMAs issue from the gpsimd queue, streams/masks from sync, so
    the ACT queue stays pure.
  - A qh-pair AllGather dedup of the k/v projections was tried and
    reverted: the DRAM-bounce collective path costs far more DMA/latency
    here than the ~30us of PE it saves.
"""

import sys

if "/opt/trn_rl_repo" not in sys.path:
    sys.path.insert(0, "/opt/trn_rl_repo")

import numpy as np

import concourse.tile as tile
from concourse import bacc, mybir
from concourse.bass_utils import run_bass_kernel_spmd

F32 = mybir.dt.float32
F16 = mybir.dt.float16
EXP = mybir.ActivationFunctionType.Exp
MULT = mybir.AluOpType.mult

B, S, D, H, R = 2, 2048, 768, 12, 8
DK = D // H  # 64
NCORES = 8
HPC = 6            # heads per core
HDIM = HPC * DK    # 384: head-slice width per core
QR = S // 2        # 1024 query rows per core
SC = 512           # streaming chunk (s dimension)
NSC = S // SC      # 4
NQC = QR // SC     # 2 query chunks per core
NKT = S // 128     # 16 key tiles
DO = D // 128      # 6 d-chunks

# of every 16 (kt-pair, head) mask multiplies, this many route to gpsimd
GPS_MASK = 0

_CACHE = {}


def _build_kernel():
    nc = bacc.Bacc("TRN2", target_bir_lowering=False, debug=False,
                   enable_asserts=True, num_devices=NCORES)

    def din(name, shape, dt=F16):
        return nc.dram_tensor(name, shape, dt, kind="ExternalInput").ap()

    qT_d = din("qT", [D, S])
    kT_d = din("kT", [D, S])
    vTh_d = din("vTh", [D, QR])
    maskT_d = din("maskT", [S, QR])
    wqt_d = din("wqt", [D, HDIM])
    wkt_d = din("wkt", [D, HDIM])
    wvt_d = din("wvt", [D, HDIM])
    wmt_d = din("wmt", [HDIM, D])
    bqc_d = din("bqc", [128, 3], F32)
    bkc_d = din("bkc", [128, 3], F32)
    out_d = nc.dram_tensor("out", [QR, D], F16, kind="ExternalOutput").ap()

    with tile.TileContext(nc) as tc:
        with (
            tc.tile_pool(name="keep", bufs=1) as keep,
            tc.tile_pool(name="maskp", bufs=2) as maskp,
            tc.tile_pool(name="wa", bufs=1) as wa,
            tc.tile_pool(name="acts", bufs=2) as acts,
            tc.tile_pool(name="bpool", bufs=2) as bpool,
            tc.tile_pool(name="epool", bufs=3) as epool,
            tc.tile_pool(name="npool", bufs=2) as npool,
            tc.tile_pool(name="fpool", bufs=2) as fpool,
            tc.tile_pool(name="psproj", bufs=2, space="PSUM") as psproj,
            tc.tile_pool(name="pss", bufs=2, space="PSUM") as pss,
            tc.tile_pool(name="pso", bufs=2, space="PSUM") as pso,
        ):
            # ---- persistent tiles ----
            wm_sb = keep.tile([128, 3, D], F16)
            kT_sb = keep.tile([128, 3, S], F16)
            qT_sb = keep.tile([128, 3, QR], F16)
            v_aug = keep.tile([128, NKT, HPC, DK + 1], F16)
            nc.vector.memset(v_aug[:, :, :, DK:DK + 1], 1.0)

            # streamed k chunk 0 first on the sync DGE so the PE can start
            kTc0 = acts.tile([128, DO, SC], F16, tag="act", name="kTc0")
            for dp in range(3):
                nc.sync.dma_start(
                    kTc0[:, 2 * dp:2 * dp + 2, :],
                    kT_d.rearrange("(o p) s -> p o s",
                                   p=128)[:, 2 * dp:2 * dp + 2, 0:SC])

            # weights ride the gpsimd DGE (Pool queue is idle; ACT stays pure)
            def wload(name, dram, shape, pat):
                t = wa.tile(shape, F16, name=name)
                nc.gpsimd.dma_start(t[:], dram.rearrange(pat, p=128))
                return t

            wk_sb = wa.tile([128, DO, HDIM], F16, name="wk_sb")
            for _et in range(3):
                _esl = slice(_et * 128, (_et + 1) * 128)
                nc.gpsimd.dma_start(
                    wk_sb[:, :, _esl],
                    wkt_d.rearrange("(o p) e -> p o e", p=128)[:, :, _esl])
            wq_sb = wload("wq_sb", wqt_d, [128, DO, HDIM], "(o p) e -> p o e")
            wv_sb = wload("wv_sb", wvt_d, [128, DO, HDIM], "(o p) e -> p o e")
            bq_sb = wa.tile([128, 3], F32)
            nc.gpsimd.dma_start(bq_sb[:], bqc_d[:])
            bk_sb = wa.tile([128, 3], F32)
            nc.gpsimd.dma_start(bk_sb[:], bkc_d[:])

            # ---- pass 1: k-projection of the OWN key-half (T-form) ----
            kTcs = {0: kTc0}

            def emit_kload(sc):
                ssl = slice(sc * SC, (sc + 1) * SC)
                kTc = acts.tile([128, DO, SC], F16, tag="act", name="kTc")
                for dp in range(3):
                    nc.sync.dma_start(
                        kTc[:, 2 * dp:2 * dp + 2, :],
                        kT_d.rearrange("(o p) s -> p o s",
                                       p=128)[:, 2 * dp:2 * dp + 2, ssl])
                kTcs[sc] = kTc

            def emit_kproj(sc, ets=(0, 1, 2)):
                ssl = slice(sc * SC, (sc + 1) * SC)
                if sc not in kTcs:
                    emit_kload(sc)
                kTc = kTcs[sc]
                for et in ets:
                    esl = slice(et * 128, (et + 1) * 128)
                    ps = psproj.tile([128, 512], F32, tag="pp", name="ps_k")
                    for do in range(DO):
                        nc.tensor.matmul(ps[:], wk_sb[:, do, esl], kTc[:, do, :],
                                         start=(do == 0), stop=(do == DO - 1))
                    nc.vector.tensor_scalar_add(
                        kT_sb[:, et, ssl], ps[:], bk_sb[:, et:et + 1])

            vThcs = {}

            def emit_qproj(qc, ets=(0, 1, 2)):
                qsl = slice(qc * SC, (qc + 1) * SC)
                if qc not in vThcs:
                    vThc = acts.tile([128, DO, SC], F16, tag="act2", name="vThc")
                    for dp in range(3):
                        nc.sync.dma_start(
                            vThc[:, 2 * dp:2 * dp + 2, :],
                            vTh_d.rearrange("(o p) s -> p o s",
                                            p=128)[:, 2 * dp:2 * dp + 2, qsl])
                    vThcs[qc] = vThc
                vThc = vThcs[qc]
                for et in ets:
                    esl = slice(et * 128, (et + 1) * 128)
                    ps = psproj.tile([128, 512], F32, tag="pp", name="ps_q")
                    for do in range(DO):
                        nc.tensor.matmul(ps[:], wq_sb[:, do, esl], vThc[:, do, :],
                                         start=(do == 0), stop=(do == DO - 1))
                    nc.vector.tensor_scalar_add(
                        qT_sb[:, et, qsl], ps[:], bq_sb[:, et:et + 1])

            # ---- phase B helpers ----
            m01s = {}
            atts = {}

            def emit_mask_load(qc):
                qsl = slice(qc * SC, (qc + 1) * SC)
                m01 = maskp.tile([128, NKT, SC], F16, tag="mb", name="m01")
                for kp in range(4):
                    nc.sync.dma_start(
                        m01[:, 4 * kp:4 * kp + 4, :],
                        maskT_d.rearrange(
                            "(o p) q -> p o q", p=128)[:, 4 * kp:4 * kp + 4, qsl])
                m01s[qc] = m01

            def emit_scores_range(qc, p, ktp_lo, ktp_hi):
                # kt-pairs: one exp tile covers 2 key tiles x 2 heads; the
                # mask multiply then runs once per pair (DVE or GPSIMD).
                qsl = slice(qc * SC, (qc + 1) * SC)
                if qc not in m01s:
                    emit_mask_load(qc)
                m01 = m01s[qc]
                if (qc, p) in atts:
                    att = atts[(qc, p)]
                else:
                    att = bpool.tile([128, NKT, 2, SC], F16, tag="att",
                                     name="att")
                    atts[(qc, p)] = att
                for ktp in range(ktp_lo, ktp_hi):
                    exp_t = epool.tile([128, 2, 2, SC], F16, tag="ex",
                                       name="exp_t")
                    for ki in range(2):
                        kt = 2 * ktp + ki
                        ps_s = pss.tile([128, 2, SC], F32, tag="ss", name="ps_s")
                        for j in range(2):
                            po = j * 64
                            nc.tensor.matmul(
                                ps_s[:, j, :],
                                kT_sb[po:po + 64, p, kt * 128:(kt + 1) * 128],
                                qT_sb[po:po + 64, p, qsl],
                                start=True, stop=True)
                        nc.scalar.activation(exp_t[:, ki, :, :], ps_s[:], EXP,
                                             scale=0.125)
                    mk = m01[:, 2 * ktp:2 * ktp + 2, :]
                    for j in range(2):
                        idx = 2 * ktp + j
                        eng = nc.gpsimd if (idx % 16) >= (16 - GPS_MASK) \
                            else nc.vector
                        eng.tensor_mul(
                            att[:, 2 * ktp:2 * ktp + 2, j, :],
                            exp_t[:, :, j, :], mk)

            def emit_scores_pair(qc, p):
                emit_scores_range(qc, p, 0, NKT // 2)

            attv_ps = {}

            def emit_attv_mms(qc, p, kt_lo, kt_hi):
                att = atts[(qc, p)]
                for j in range(2):
                    h = 2 * p + j
                    if (qc, p, j) not in attv_ps:
                        attv_ps[(qc, p, j)] = pso.tile([DK + 1, SC], F32,
                                                       tag="oo", name="ps_o")
                    ps_o = attv_ps[(qc, p, j)]
                    for kt in range(kt_lo, kt_hi):
                        nc.tensor.matmul(ps_o[:], v_aug[:, kt, h, :],
                                         att[:, kt, j, :],
                                         start=(kt == 0), stop=(kt == NKT - 1))

            def emit_attv_epilogue(qc, p):
                atts.pop((qc, p))
                for j in range(2):
                    po = j * 64
                    ps_o = attv_ps.pop((qc, p, j))
                    den_sb = npool.tile([1, SC], F32, tag="den", name="den_sb")
                    nc.vector.tensor_copy(den_sb[:], ps_o[DK:DK + 1, :])
                    r_sb = npool.tile([1, SC], F32, tag="r", name="r_sb")
                    nc.vector.reciprocal_approx_fast(r_sb[:], den_sb[:])
                    bb = npool.tile([64, SC], F32, tag="bb", name="bb")
                    nc.gpsimd.partition_broadcast(bb[:], r_sb[:])
                    if 2 * p + j == 0:
                        _OUTT[qc] = bpool.tile([128, 3, SC], F16, tag="outT",
                                               name="outT")
                    outT_sb = _OUTT[qc]
                    nc.vector.tensor_mul(outT_sb[po:po + 64, p, :],
                                         ps_o[0:DK, :], bb[:])

            def emit_attv_pair(qc, p):
                emit_attv_mms(qc, p, 0, NKT)
                emit_attv_epilogue(qc, p)

            def emit_merge(qc, qts=(0, 1, 2, 3)):
                outT_sb = _OUTT[qc]
                for qt in qts:
                    qtsl = slice(qt * 128, (qt + 1) * 128)
                    for ec in range(2):
                        esl = slice(ec * 384, (ec + 1) * 384)
                        ps_m = psproj.tile([128, 512], F32, tag="pp", name="ps_m")
                        for hp in range(3):
                            nc.tensor.matmul(ps_m[:, :384], outT_sb[:, hp, qtsl],
                                             wm_sb[:, hp, esl],
                                             start=(hp == 0), stop=(hp == 2))
                        fin = fpool.tile([128, 384], F16, tag="fin", name="fin")
                        nc.vector.tensor_copy(fin[:], ps_m[:, :384])
                        nc.sync.dma_start(
                            out_d[qc * SC + qt * 128:qc * SC + (qt + 1) * 128, esl],
                            fin[:])

            # ---- pass 2: v-projection of the OWN key-half ----
            def emit_pass2(sc):
                ssl = slice(sc * SC, (sc + 1) * SC)
                qTc = acts.tile([128, DO, SC], F16, tag="act2", name="qTc")
                for dp in range(3):
                    nc.sync.dma_start(
                        qTc[:, 2 * dp:2 * dp + 2, :],
                        qT_d.rearrange("(o p) s -> p o s",
                                       p=128)[:, 2 * dp:2 * dp + 2, ssl])
                for st in range(4):
                    gst = sc * 4 + st
                    stsl = slice(st * 128, (st + 1) * 128)
                    ps = psproj.tile([128, 512], F32, tag="pp", name="ps_v")
                    for do in range(DO):
                        nc.tensor.matmul(ps[:, :HDIM], qTc[:, do, stsl],
                                         wv_sb[:, do, :],
                                         start=(do == 0), stop=(do == DO - 1))
                    nc.vector.tensor_copy(
                        v_aug[:, gst, :, 0:DK],
                        ps[:, :HDIM].rearrange("p (h d) -> p h d", h=HPC))

            # pass 1 with scores(0,0) interleaved: the exp chain starts
            # during the DMA-paced k-projection.
            emit_kproj(0)
            emit_qproj(0)
            emit_scores_range(0, 0, 0, 2)
            emit_kproj(1)
            emit_scores_range(0, 0, 2, 4)
            emit_kproj(2)
            emit_scores_range(0, 0, 4, 6)
            emit_qproj(1)
            emit_kproj(3)
            emit_scores_range(0, 0, 6, 8)
            emit_pass2(0)
            emit_scores_pair(1, 0)
            emit_pass2(1)
            emit_pass2(2)
            emit_pass2(3)

            # merge weights needed only at the end; gpsimd DGE
            nc.gpsimd.dma_start(wm_sb[:],
                                wmt_d.rearrange("(o p) e -> p o e", p=128))

            # ---- pipelined attention tail ----
            emit_attv_pair(0, 0)
            emit_scores_pair(0, 1)
            emit_attv_pair(1, 0)
            emit_scores_pair(1, 1)
            emit_attv_pair(0, 1)
            emit_scores_pair(0, 2)
            emit_attv_pair(1, 1)
            emit_scores_pair(1, 2)
            emit_attv_pair(0, 2)
            emit_merge(0)
            emit_attv_pair(1, 2)
            emit_merge(1)

    nc.compile()
    return nc


_OUTT = {}


def _shard_inputs(inputs):
    q = np.asarray(inputs["query"], np.float32)
    k = np.asarray(inputs["key"], np.float32)
    v = np.asarray(inputs["value"], np.float32)
    mask = np.asarray(inputs["mask"], np.int32)
    Wq = np.asarray(inputs["Wq"], np.float32)
    Wk = np.asarray(inputs["Wk"], np.float32)
    Wv = np.asarray(inputs["Wv"], np.float32)
    Wm = np.asarray(inputs["Wm"], np.float32)
    bq = np.asarray(inputs["bq"], np.float32)
    bk = np.asarray(inputs["bk"], np.float32)
    Aq = np.asarray(inputs["lora_A_q"], np.float32)
    Bq = np.asarray(inputs["lora_B_q"], np.float32)
    Av = np.asarray(inputs["lora_A_v"], np.float32)
    Bv = np.asarray(inputs["lora_B_v"], np.float32)

    def c(x):
        return np.ascontiguousarray(x)

    def ch(x):
        return np.ascontiguousarray(x.astype(np.float16))

    # fold the rank-8 LoRA deltas on the host (cheap, exact)
    Qadj = q + (q @ Aq) @ Bq      # feeds the v-projection stream
    Vadj = v + (v @ Av) @ Bv      # feeds the q-projection stream

    qT = [ch(Qadj[b].T) for b in range(B)]
    kT = [ch(k[b].T) for b in range(B)]
    vT = [ch(Vadj[b].T) for b in range(B)]
    mT = [ch(mask[b].T) for b in range(B)]
    WqT, WkT, WvT, WmT = ch(Wq.T), ch(Wk.T), ch(Wv.T), ch(Wm.T)

    in_maps = []
    for core in range(NCORES):
        b, qh, hh = core // 4, (core // 2) % 2, core % 2
        hsl = slice(hh * HDIM, (hh + 1) * HDIM)
        qrows = slice(qh * QR, (qh + 1) * QR)
        in_maps.append({
            "qT": qT[b],
            "kT": kT[b],
            "vTh": c(vT[b][:, qrows]),
            "maskT": c(mT[b][:, qrows]),
            "wqt": c(WqT[:, hsl]),
            "wkt": c(WkT[:, hsl]),
            "wvt": c(WvT[:, hsl]),
            "wmt": c(WmT[hsl, :]),
            "bqc": c(bq[hsl].reshape(3, 128).T),
            "bkc": c(bk[hsl].reshape(3, 128).T),
        })
    return in_maps


def _get_nc():
    if "nc" not in _CACHE:
        _CACHE["nc"] = _build_kernel()
    return _CACHE["nc"]


def kernel(**inputs) -> np.ndarray:
    nc = _get_nc()
    in_maps = _shard_inputs(inputs)
    res = run_bass_kernel_spmd(nc, in_maps, core_ids=list(range(NCORES)))
    # bv is constant along keys, so it passes through the softmax average
    # exactly; fold it (and bm) into a host-side output bias.
    bm = np.asarray(inputs["bm"], np.float32)
    bv = np.asarray(inputs["bv"], np.float32)
    Wm = np.asarray(inputs["Wm"], np.float32)
    bm_eff = bm + bv @ Wm.T
    out = np.zeros((B, S, D), np.float32)
    for b in range(B):
        for qh in range(2):
            part = (np.asarray(res.results[b * 4 + qh * 2 + 0]["out"],
                               np.float32)
                    + np.asarray(res.results[b * 4 + qh * 2 + 1]["out"],
                                 np.float32))
            out[b, qh * QR:(qh + 1) * QR, :] = part + bm_eff[None, :]
    return out


# revision 37
# speedup vs baseline: 1.2194x; 1.0787x over previous
"""Trainium2 Bass kernel for LoRA multi-head attention (B=2, S=2048, D=768, H=12, R=8).

Sharding over 8 cores: (batch, query-half, head-half) -> each core computes
6 heads x 1024 query rows x full 2048 keys, producing a partial (over the
head dimension) of the final merge projection. Host sums the two head-half
partials per (batch, query-half) slice and adds the merge bias.

Host-side folding (all exact):
  - LoRA deltas are rank-8: folded on the host; the device sees adjusted
    streams.
  - bv is constant per head-dim: folded into the output bias as
    bm_eff = bm + bv @ Wm.T.

Engine budget per core (the design):
  - ACT (scalar) engine does ONLY the 96 exp instructions (the hard floor,
    ~100us busy). No copies, no DMA issue on its queue.
  - fp16 everywhere instead of bf16: PE speed is identical and the error
    drops ~10x (8.5e-4 vs 8.2e-3), far under the 2e-2 gate. fp8/DoubleRow
    was measured numerically infeasible for this gate (3-6e-2).
  - scores pack the two heads of a pair into the 64-row PE halves
    (row-tiled matmuls overlap on the array).
  - att@v keeps v stationary (N=512 streams; the flipped att-stationary
    layout loses to per-matmul LDWEIGHTS cost). Epilogue: den copy +
    fast reciprocal, gpsimd partition-broadcast, one fused
    normalize-multiply into outT (the separate ob-copy is folded in).
  - weight DMAs issue from the gpsimd queue, streams/masks from sync, so
    the ACT queue stays pure.
  - A qh-pair AllGather dedup of the k/v projections was tried and
    reverted: the DRAM-bounce collective path costs far more DMA/latency
    here than the ~30us of PE it saves.
"""

import sys

if "/opt/trn_rl_repo" not in sys.path:
    sys.path.insert(0, "/opt/trn_rl_repo")

import numpy as np

import concourse.tile as tile
from concourse import bacc, mybir
from concourse.bass_utils import run_bass_kernel_spmd

F32 = mybir.dt.float32
F16 = mybir.dt.float16
EXP = mybir.ActivationFunctionType.Exp
MULT = mybir.AluOpType.mult

B, S, D, H, R = 2, 2048, 768, 12, 8
DK = D // H  # 64
NCORES = 8
HPC = 6            # heads per core
HDIM = HPC * DK    # 384: head-slice width per core
QR = S // 2        # 1024 query rows per core
SC = 512           # streaming chunk (s dimension)
NSC = S // SC      # 4
NQC = QR // SC     # 2 query chunks per core
NKT = S // 128     # 16 key tiles
DO = D // 128      # 6 d-chunks

# of every 16 (kt-pair, head) mask multiplies, this many route to gpsimd
GPS_MASK = 0

_CACHE = {}


def _build_kernel():
    nc = bacc.Bacc("TRN2", target_bir_lowering=False, debug=False,
                   enable_asserts=True, num_devices=NCORES)

    def din(name, shape, dt=F16):
        return nc.dram_tensor(name, shape, dt, kind="ExternalInput").ap()

    qT_d = din("qT", [D, S])
    kT_d = din("kT", [D, S])
    vTh_d = din("vTh", [D, QR])
    maskT_d = din("maskT", [S, QR])
    wqt_d = din("wqt", [D, HDIM])
    wkt_d = din("wkt", [D, HDIM])
    wvt_d = din("wvt", [D, HDIM])
    wmt_d = din("wmt", [HDIM, D])
    bqc_d = din("bqc", [128, 3], F32)
    bkc_d = din("bkc", [128, 3], F32)
    out_d = nc.dram_tensor("out", [QR, D], F16, kind="ExternalOutput").ap()

    with tile.TileContext(nc) as tc:
        with (
            tc.tile_pool(name="keep", bufs=1) as keep,
            tc.tile_pool(name="maskp", bufs=2) as maskp,
            tc.tile_pool(name="wa", bufs=1) as wa,
            tc.tile_pool(name="acts", bufs=2) as acts,
            tc.tile_pool(name="bpool", bufs=2) as bpool,
            tc.tile_pool(name="epool", bufs=2) as epool,
            tc.tile_pool(name="npool", bufs=2) as npool,
            tc.tile_pool(name="fpool", bufs=2) as fpool,
            tc.tile_pool(name="psproj", bufs=2, space="PSUM") as psproj,
            tc.tile_pool(name="pss", bufs=2, space="PSUM") as pss,
            tc.tile_pool(name="pso", bufs=2, space="PSUM") as pso,
        ):
            # ---- persistent tiles ----
            wm_sb = keep.tile([128, 3, D], F16)
            kT_sb = keep.tile([128, 3, S], F16)
            qT_sb = keep.tile([128, 3, QR], F16)
            v_aug = keep.tile([128, NKT, HPC, DK + 1], F16)
            nc.vector.memset(v_aug[:, :, :, DK:DK + 1], 1.0)

            # streamed k chunk 0 first on the sync DGE so the PE can start
            kTc0 = acts.tile([128, DO, SC], F16, tag="act", name="kTc0")
            nc.sync.dma_start(
                kTc0[:],
                kT_d.rearrange("(o p) s -> p o s", p=128)[:, :, 0:SC])

            # weights ride the gpsimd DGE (Pool queue is idle; ACT stays pure)
            def wload(name, dram, shape, pat):
                t = wa.tile(shape, F16, name=name)
                nc.gpsimd.dma_start(t[:], dram.rearrange(pat, p=128))
                return t

            wk_sb = wa.tile([128, DO, HDIM], F16, name="wk_sb")
            for _et in range(3):
                _esl = slice(_et * 128, (_et + 1) * 128)
                nc.gpsimd.dma_start(
                    wk_sb[:, :, _esl],
                    wkt_d.rearrange("(o p) e -> p o e", p=128)[:, :, _esl])
            wq_sb = wload("wq_sb", wqt_d, [128, DO, HDIM], "(o p) e -> p o e")
            wv_sb = wload("wv_sb", wvt_d, [128, DO, HDIM], "(o p) e -> p o e")
            bq_sb = wa.tile([128, 3], F32)
            nc.gpsimd.dma_start(bq_sb[:], bqc_d[:])
            bk_sb = wa.tile([128, 3], F32)
            nc.gpsimd.dma_start(bk_sb[:], bkc_d[:])

            # ---- pass 1: k-projection of the OWN key-half (T-form) ----
            kTcs = {0: kTc0}

            def emit_kload(sc):
                ssl = slice(sc * SC, (sc + 1) * SC)
                kTc = acts.tile([128, DO, SC], F16, tag="act", name="kTc")
                nc.sync.dma_start(
                    kTc[:],
                    kT_d.rearrange("(o p) s -> p o s", p=128)[:, :, ssl])
                kTcs[sc] = kTc

            def emit_kproj(sc, ets=(0, 1, 2)):
                ssl = slice(sc * SC, (sc + 1) * SC)
                if sc not in kTcs:
                    emit_kload(sc)
                kTc = kTcs[sc]
                for et in ets:
                    esl = slice(et * 128, (et + 1) * 128)
                    ps = psproj.tile([128, 512], F32, tag="pp", name="ps_k")
                    for do in range(DO):
                        nc.tensor.matmul(ps[:], wk_sb[:, do, esl], kTc[:, do, :],
                                         start=(do == 0), stop=(do == DO - 1))
                    nc.vector.tensor_scalar_add(
                        kT_sb[:, et, ssl], ps[:], bk_sb[:, et:et + 1])

            vThcs = {}

            def emit_qproj(qc, ets=(0, 1, 2)):
                qsl = slice(qc * SC, (qc + 1) * SC)
                if qc not in vThcs:
                    vThc = acts.tile([128, DO, SC], F16, tag="act2", name="vThc")
                    nc.sync.dma_start(
                        vThc[:],
                        vTh_d.rearrange("(o p) s -> p o s", p=128)[:, :, qsl])
                    vThcs[qc] = vThc
                vThc = vThcs[qc]
                for et in ets:
                    esl = slice(et * 128, (et + 1) * 128)
                    ps = psproj.tile([128, 512], F32, tag="pp", name="ps_q")
                    for do in range(DO):
                        nc.tensor.matmul(ps[:], wq_sb[:, do, esl], vThc[:, do, :],
                                         start=(do == 0), stop=(do == DO - 1))
                    nc.vector.tensor_scalar_add(
                        qT_sb[:, et, qsl], ps[:], bq_sb[:, et:et + 1])

            # ---- phase B helpers ----
            m01s = {}
            atts = {}

            def emit_mask_load(qc):
                qsl = slice(qc * SC, (qc + 1) * SC)
                m01 = maskp.tile([128, NKT, SC], F16, tag="mb", name="m01")
                for kp in range(4):
                    nc.sync.dma_start(
                        m01[:, 4 * kp:4 * kp + 4, :],
                        maskT_d.rearrange(
                            "(o p) q -> p o q", p=128)[:, 4 * kp:4 * kp + 4, qsl])
                m01s[qc] = m01

            def emit_scores_range(qc, p, ktp_lo, ktp_hi):
                # kt-pairs: one exp tile covers 2 key tiles x 2 heads; the
                # mask multiply then runs once per pair (DVE or GPSIMD).
                qsl = slice(qc * SC, (qc + 1) * SC)
                if qc not in m01s:
                    emit_mask_load(qc)
                m01 = m01s[qc]
                if (qc, p) in atts:
                    att = atts[(qc, p)]
                else:
                    att = bpool.tile([128, NKT, 2, SC], F16, tag="att",
                                     name="att")
                    atts[(qc, p)] = att
                for ktp in range(ktp_lo, ktp_hi):
                    exp_t = epool.tile([128, 2, 2, SC], F16, tag="ex",
                                       name="exp_t")
                    for ki in range(2):
                        kt = 2 * ktp + ki
                        ps_s = pss.tile([128, 2, SC], F32, tag="ss", name="ps_s")
                        for j in range(2):
                            po = j * 64
                            nc.tensor.matmul(
                                ps_s[:, j, :],
                                kT_sb[po:po + 64, p, kt * 128:(kt + 1) * 128],
                                qT_sb[po:po + 64, p, qsl],
                                start=True, stop=True)
                        nc.scalar.activation(exp_t[:, ki, :, :], ps_s[:], EXP,
                                             scale=0.125)
                    mk = m01[:, 2 * ktp:2 * ktp + 2, :]
                    for j in range(2):
                        idx = 2 * ktp + j
                        eng = nc.gpsimd if (idx % 16) >= (16 - GPS_MASK) \
                            else nc.vector
                        eng.tensor_mul(
                            att[:, 2 * ktp:2 * ktp + 2, j, :],
                            exp_t[:, :, j, :], mk)

            def emit_scores_pair(qc, p):
                emit_scores_range(qc, p, 0, NKT // 2)

            attv_ps = {}

            def emit_attv_mms(qc, p, kt_lo, kt_hi):
                att = atts[(qc, p)]
                for j in range(2):
                    h = 2 * p + j
                    if (qc, p, j) not in attv_ps:
                        attv_ps[(qc, p, j)] = pso.tile([DK + 1, SC], F32,
                                                       tag="oo", name="ps_o")
                    ps_o = attv_ps[(qc, p, j)]
                    for kt in range(kt_lo, kt_hi):
                        nc.tensor.matmul(ps_o[:], v_aug[:, kt, h, :],
                                         att[:, kt, j, :],
                                         start=(kt == 0), stop=(kt == NKT - 1))

            def emit_attv_epilogue(qc, p):
                atts.pop((qc, p))
                for j in range(2):
                    po = j * 64
                    ps_o = attv_ps.pop((qc, p, j))
                    den_sb = npool.tile([1, SC], F32, tag="den", name="den_sb")
                    nc.vector.tensor_copy(den_sb[:], ps_o[DK:DK + 1, :])
                    r_sb = npool.tile([1, SC], F32, tag="r", name="r_sb")
                    nc.vector.reciprocal_approx_fast(r_sb[:], den_sb[:])
                    bb = npool.tile([64, SC], F32, tag="bb", name="bb")
                    nc.gpsimd.partition_broadcast(bb[:], r_sb[:])
                    if 2 * p + j == 0:
                        _OUTT[qc] = bpool.tile([128, 3, SC], F16, tag="outT",
                                               name="outT")
                    outT_sb = _OUTT[qc]
                    nc.vector.tensor_mul(outT_sb[po:po + 64, p, :],
                                         ps_o[0:DK, :], bb[:])

            def emit_attv_pair(qc, p):
                emit_attv_mms(qc, p, 0, NKT)
                emit_attv_epilogue(qc, p)

            def emit_merge(qc, qts=(0, 1, 2, 3)):
                outT_sb = _OUTT[qc]
                for qt in qts:
                    qtsl = slice(qt * 128, (qt + 1) * 128)
                    for ec in range(2):
                        esl = slice(ec * 384, (ec + 1) * 384)
                        ps_m = psproj.tile([128, 512], F32, tag="pp", name="ps_m")
                        for hp in range(3):
                            nc.tensor.matmul(ps_m[:, :384], outT_sb[:, hp, qtsl],
                                             wm_sb[:, hp, esl],
                                             start=(hp == 0), stop=(hp == 2))
                        fin = fpool.tile([128, 384], F16, tag="fin", name="fin")
                        nc.vector.tensor_copy(fin[:], ps_m[:, :384])
                        nc.sync.dma_start(
                            out_d[qc * SC + qt * 128:qc * SC + (qt + 1) * 128, esl],
                            fin[:])

            # ---- pass 2: v-projection of the OWN key-half ----
            def emit_pass2(sc):
                ssl = slice(sc * SC, (sc + 1) * SC)
                qTc = acts.tile([128, DO, SC], F16, tag="act2", name="qTc")
                nc.sync.dma_start(
                    qTc[:],
                    qT_d.rearrange("(o p) s -> p o s", p=128)[:, :, ssl])
                for st in range(4):
                    gst = sc * 4 + st
                    stsl = slice(st * 128, (st + 1) * 128)
                    ps = psproj.tile([128, 512], F32, tag="pp", name="ps_v")
                    for do in range(DO):
                        nc.tensor.matmul(ps[:, :HDIM], qTc[:, do, stsl],
                                         wv_sb[:, do, :],
                                         start=(do == 0), stop=(do == DO - 1))
                    nc.vector.tensor_copy(
                        v_aug[:, gst, :, 0:DK],
                        ps[:, :HDIM].rearrange("p (h d) -> p h d", h=HPC))

            # pass 1 with scores(0,0) interleaved: the exp chain starts
            # during the DMA-paced k-projection.
            emit_kproj(0)
            emit_qproj(0)
            emit_kload(1)
            emit_scores_range(0, 0, 0, 2)
            emit_kproj(1)
            emit_scores_range(0, 0, 2, 4)
            emit_kproj(2)
            emit_scores_range(0, 0, 4, 6)
            emit_qproj(1)
            emit_kproj(3)
            emit_scores_range(0, 0, 6, 8)
            emit_pass2(0)
            emit_scores_pair(1, 0)
            emit_pass2(1)
            emit_pass2(2)
            emit_pass2(3)

            # merge weights needed only at the end; gpsimd DGE
            nc.gpsimd.dma_start(wm_sb[:],
                                wmt_d.rearrange("(o p) e -> p o e", p=128))

            # ---- pipelined attention tail ----
            emit_attv_pair(0, 0)
            emit_scores_pair(0, 1)
            emit_attv_pair(1, 0)
            emit_scores_pair(1, 1)
            emit_attv_pair(0, 1)
            emit_scores_pair(0, 2)
            emit_attv_pair(1, 1)
            emit_scores_pair(1, 2)
            emit_attv_pair(0, 2)
            emit_merge(0)
            emit_attv_pair(1, 2)
            emit_merge(1)

    nc.compile()
    return nc


_OUTT = {}


def _shard_inputs(inputs):
    q = np.asarray(inputs["query"], np.float32)
    k = np.asarray(inputs["key"], np.float32)
    v = np.asarray(inputs["value"], np.float32)
    mask = np.asarray(inputs["mask"], np.int32)
    Wq = np.asarray(inputs["Wq"], np.float32)
    Wk = np.asarray(inputs["Wk"], np.float32)
    Wv = np.asarray(inputs["Wv"], np.float32)
    Wm = np.asarray(inputs["Wm"], np.float32)
    bq = np.asarray(inputs["bq"], np.float32)
    bk = np.asarray(inputs["bk"], np.float32)
    Aq = np.asarray(inputs["lora_A_q"], np.float32)
    Bq = np.asarray(inputs["lora_B_q"], np.float32)
    Av = np.asarray(inputs["lora_A_v"], np.float32)
    Bv = np.asarray(inputs["lora_B_v"], np.float32)

    def c(x):
        return np.ascontiguousarray(x)

    def ch(x):
        return np.ascontiguousarray(x.astype(np.float16))

    # fold the rank-8 LoRA deltas on the host (cheap, exact)
    Qadj = q + (q @ Aq) @ Bq      # feeds the v-projection stream
    Vadj = v + (v @ Av) @ Bv      # feeds the q-projection stream

    qT = [ch(Qadj[b].T) for b in range(B)]
    kT = [ch(k[b].T) for b in range(B)]
    vT = [ch(Vadj[b].T) for b in range(B)]
    mT = [ch(mask[b].T) for b in range(B)]
    WqT, WkT, WvT, WmT = ch(Wq.T), ch(Wk.T), ch(Wv.T), ch(Wm.T)

    in_maps = []
    for core in range(NCORES):
        b, qh, hh = core // 4, (core // 2) % 2, core % 2
        hsl = slice(hh * HDIM, (hh + 1) * HDIM)
        qrows = slice(qh * QR, (qh + 1) * QR)
        in_maps.append({
            "qT": qT[b],
            "kT": kT[b],
            "vTh": c(vT[b][:, qrows]),
            "maskT": c(mT[b][:, qrows]),
            "wqt": c(WqT[:, hsl]),
            "wkt": c(WkT[:, hsl]),
            "wvt": c(WvT[:, hsl]),
            "wmt": c(WmT[hsl, :]),
            "bqc": c(bq[hsl].reshape(3, 128).T),
            "bkc": c(bk[hsl].reshape(3, 128).T),
        })
    return in_maps


def _get_nc():
    if "nc" not in _CACHE:
        _CACHE["nc"] = _build_kernel()
    return _CACHE["nc"]


def kernel(**inputs) -> np.ndarray:
    nc = _get_nc()
    in_maps = _shard_inputs(inputs)
    res = run_bass_kernel_spmd(nc, in_maps, core_ids=list(range(NCORES)))
    # bv is constant along keys, so it passes through the softmax average
    # exactly; fold it (and bm) into a host-side output bias.
    bm = np.asarray(inputs["bm"], np.float32)
    bv = np.asarray(inputs["bv"], np.float32)
    Wm = np.asarray(inputs["Wm"], np.float32)
    bm_eff = bm + bv @ Wm.T
    out = np.zeros((B, S, D), np.float32)
    for b in range(B):
        for qh in range(2):
            part = (np.asarray(res.results[b * 4 + qh * 2 + 0]["out"],
                               np.float32)
                    + np.asarray(res.results[b * 4 + qh * 2 + 1]["out"],
                                 np.float32))
            out[b, qh * QR:(qh + 1) * QR, :] = part + bm_eff[None, :]
    return out
